# revision 7
# baseline (speedup 1.0000x reference)
"""Trainium2 Bass kernel for nn_Attention_New_14431090114891.

Computation (B=32, S=1024, H=1024, E=512), per batch sample:
    x     = d @ W_in + b_in                      # linearInput
    q     = x + g                                # decoderstate (pre-scale)
    sc    = (q * sqrt(.5)) @ z^T                 # attention scores [S, S]
    attn  = softmax(sc, axis=-1)
    cond  = attn @ c * sqrt(S)
    out   = ((x + cond) * sqrt(.5)) @ W_out + b_out

Strategy: data-parallel over batch, 4 samples per core on 8 NeuronCores.
All heavy matmuls run as float32r (FP22 multiply, fp32 accumulate) at full
PE rate.  The whole per-sample pipeline works in "feature-major" [E, S]
layout so that every matmul's contraction dim lands on SBUF partitions:

    xT [E,S] = W_in(lhsT, natural) . dT          (d transposed on PE)
    qT       = xT + gT                           (g transposed on PE)
    scores   = qT(lhsT) . (z^T * sqrt(.5))       -> [s-part, t-free]
    softmax along free axis (DVE max, ACT exp with accum rowsum)
    attn_n   = exp * (sqrt(S)/rowsum)            (DVE, in place)
    condT    = c(lhsT, natural) . attn_n^T       (attn transposed on PE)
    out2T    = condT + xT                        (residual, in place into xT)
    final    = out2T(lhsT) . (W_out * sqrt(.5))  -> [s-part, h-free] -> DRAM

sqrt(.5) folds into the z-transpose eviction and a host-side W_out
prescale; sqrt(S) folds into the softmax normalizer; b_in is applied as a
per-partition ACT bias during the xT eviction; b_out is added on the host
(it is all-zeros in practice, so that add is skipped).
"""

import math
from contextlib import ExitStack

import numpy as np

import concourse.bass as bass
import concourse.mybir as mybir
import concourse.tile as tile
from concourse import bacc, bass_utils
from concourse.masks import make_identity

# Problem shapes (hardcoded per contract).
B, S, H, E = 32, 1024, 1024, 512
N_CORES = 8
BPC = B // N_CORES          # samples per core
SBLK = 512                  # s-block (free-dim N of most matmuls)
NSBLK = S // SBLK           # 2 blocks per sample
NSUB = SBLK // 128          # 4 s-subtiles of 128 per block
HT, ET, TT = H // 128, E // 128, S // 128   # partition-tile counts
SQRT_HALF = float(np.sqrt(0.5))
SQRT_S = float(np.sqrt(float(S)))

F32 = mybir.dt.float32
F32R = mybir.dt.float32r


def _r(ap):
    """Matmul-operand tiles are already float32r; keep as passthrough."""
    return ap


def build_program():
    nc = bacc.Bacc("TRN2", target_bir_lowering=False, debug=False)

    d_dram = nc.dram_tensor("d", [BPC, S, H], F32, kind="ExternalInput").ap()
    g_dram = nc.dram_tensor("g", [BPC, S, E], F32, kind="ExternalInput").ap()
    z_dram = nc.dram_tensor("z", [BPC, S, E], F32, kind="ExternalInput").ap()
    c_dram = nc.dram_tensor("c", [BPC, S, E], F32R, kind="ExternalInput").ap()
    win_dram = nc.dram_tensor("win", [H, E], F32R, kind="ExternalInput").ap()
    wout_dram = nc.dram_tensor("wout_s", [E, H], F32R, kind="ExternalInput").ap()
    bin_dram = nc.dram_tensor("bin_t", [128, ET], F32, kind="ExternalInput").ap()
    out_dram = nc.dram_tensor("out", [BPC, S, H], F32, kind="ExternalOutput").ap()

    with tile.TileContext(nc) as tc, ExitStack() as ctx:
        consts = ctx.enter_context(tc.tile_pool(name="consts", bufs=1))
        samp = ctx.enter_context(tc.tile_pool(name="samp", bufs=1))
        cpool = ctx.enter_context(tc.tile_pool(name="cpool", bufs=2))
        blk = ctx.enter_context(tc.tile_pool(name="blk", bufs=1))
        stage = ctx.enter_context(tc.tile_pool(name="stage", bufs=2))
        sm = ctx.enter_context(tc.tile_pool(name="sm", bufs=2))
        ps_mm = ctx.enter_context(tc.tile_pool(name="ps_mm", bufs=2, space="PSUM"))
        ps_sc = ctx.enter_context(tc.tile_pool(name="ps_sc", bufs=2, space="PSUM"))
        ps_tr = ctx.enter_context(tc.tile_pool(name="ps_tr", bufs=2, space="PSUM"))

        # ---- constants ----
        ident = consts.tile([128, 128], F32)
        make_identity(nc, ident)
        win_sb = consts.tile([128, HT, E], F32R)        # [h-part, h-tile, e]
        nc.sync.dma_start(out=win_sb, in_=win_dram.rearrange("(ht p) e -> p ht e", p=128))
        wout_sb = consts.tile([128, ET, H], F32R)       # [e-part, e-tile, h]
        nc.sync.dma_start(out=wout_sb, in_=wout_dram.rearrange("(et p) h -> p et h", p=128))
        bin_sb = consts.tile([128, ET], F32)           # b_in, e-partition layout
        nc.sync.dma_start(out=bin_sb, in_=bin_dram)

        def transpose_group(src_fn, n, psum_dtype=F32):
            """Transpose `n` (<=4) [128,128] SBUF slices into one PSUM bank.

            src_fn(k) -> AP of the k-th input slice.  Returns the PSUM tile
            [128, n*128] holding the transposed slices side by side.
            """
            pt = ps_tr.tile([128, 512], psum_dtype, tag="tr")
            for k in range(n):
                nc.tensor.transpose(pt[:, k * 128:(k + 1) * 128], src_fn(k), ident)
            return pt

        for smp in range(BPC):
            # ---- per-sample: z^T (scaled) and c ----
            z_stage = samp.tile([128, TT, E], F32)     # [t-part, t-tile, e]
            nc.sync.dma_start(out=z_stage, in_=z_dram[smp].rearrange("(tt p) e -> p tt e", p=128))
            zsT = samp.tile([128, ET, S], F32R)         # z^T * sqrt(.5) [e-part, e-tile, t]
            for et in range(ET):
                for tt0 in range(0, TT, 4):
                    pt = transpose_group(
                        lambda k: z_stage[:, tt0 + k, et * 128:(et + 1) * 128], 4)
                    nc.scalar.activation(
                        out=zsT[:, et, tt0 * 128:(tt0 + 4) * 128], in_=pt,
                        func=mybir.ActivationFunctionType.Copy, scale=SQRT_HALF)
            c_sb = cpool.tile([128, TT, E], F32R)       # [t-part, t-tile, e] natural
            nc.sync.dma_start(out=c_sb, in_=c_dram[smp].rearrange("(tt p) e -> p tt e", p=128))

            for b in range(NSBLK):
                s0 = b * SBLK

                # ---- dT: transpose the d block ----
                dT = blk.tile([128, HT, SBLK], F32R)    # [h-part, h-tile, s]
                for j in range(NSUB):
                    d_raw = stage.tile([128, H], F32)
                    nc.sync.dma_start(out=d_raw, in_=d_dram[smp, s0 + j * 128: s0 + (j + 1) * 128, :])
                    for ht0 in range(0, HT, 4):
                        pt = transpose_group(
                            lambda k: d_raw[:, (ht0 + k) * 128:(ht0 + k + 1) * 128], 4)
                        nc.scalar.copy(
                            out=dT[:, ht0:ht0 + 4, j * 128:(j + 1) * 128],
                            in_=pt.rearrange("p (a b) -> p a b", a=4))

                # ---- gT -> qT (add xT later) ----
                qT = blk.tile([128, ET, SBLK], F32R)    # [e-part, e-tile, s]
                for j in range(NSUB):
                    g_raw = stage.tile([128, E], F32)
                    nc.sync.dma_start(out=g_raw, in_=g_dram[smp, s0 + j * 128: s0 + (j + 1) * 128, :])
                    pt = transpose_group(
                        lambda k: g_raw[:, k * 128:(k + 1) * 128], ET)
                    nc.vector.tensor_copy(
                        out=qT[:, :, j * 128:(j + 1) * 128],
                        in_=pt.rearrange("p (a b) -> p a b", a=ET))

                # ---- xT = W_in^T . dT (+ b_in), then qT += xT ----
                xT = blk.tile([128, ET, SBLK], F32R)
                for et in range(ET):
                    pm = ps_mm.tile([128, SBLK], F32, tag="mm")
                    for ht in range(HT):
                        nc.tensor.matmul(
                            pm, _r(win_sb[:, ht, et * 128:(et + 1) * 128]),
                            _r(dT[:, ht, :]), start=(ht == 0), stop=(ht == HT - 1))
                    nc.scalar.activation(
                        out=xT[:, et, :], in_=pm,
                        func=mybir.ActivationFunctionType.Identity,
                        bias=bin_sb[:, et:et + 1], scale=1.0)
                    nc.vector.tensor_add(out=qT[:, et, :], in0=qT[:, et, :], in1=xT[:, et, :])

                # ---- per s-subtile: scores, softmax, attn^T ----
                attnT = blk.tile([128, TT, SBLK], F32R)  # [t-part, t-tile, s]
                for j in range(NSUB):
                    psc = ps_sc.tile([128, S], F32, tag="sc")
                    for th in range(S // 512):
                        for et in range(ET):
                            nc.tensor.matmul(
                                psc[:, th * 512:(th + 1) * 512],
                                _r(qT[:, et, j * 128:(j + 1) * 128]),
                                _r(zsT[:, et, th * 512:(th + 1) * 512]),
                                start=(et == 0), stop=(et == ET - 1))
                    negmax = sm.tile([128, 1], F32)
                    nc.vector.reduce_max(negmax, psc, axis=mybir.AxisListType.X, negate=True)
                    rowsum = sm.tile([128, 1], F32)
                    expsc = sm.tile([128, S], F32)
                    nc.scalar.activation(
                        out=expsc, in_=psc, func=mybir.ActivationFunctionType.Exp,
                        bias=negmax, scale=1.0, accum_out=rowsum)
                    recip = sm.tile([128, 1], F32)
                    nc.vector.reciprocal(recip, rowsum)
                    # attn * sqrt(S) / rowsum, in place
                    nc.vector.tensor_scalar(
                        out=expsc, in0=expsc, scalar1=recip, scalar2=SQRT_S,
                        op0=mybir.AluOpType.mult, op1=mybir.AluOpType.mult)
                    for tt0 in range(0, TT, 4):
                        pt = transpose_group(
                            lambda k: expsc[:, (tt0 + k) * 128:(tt0 + k + 1) * 128], 4)
                        nc.vector.tensor_copy(
                            out=attnT[:, tt0:tt0 + 4, j * 128:(j + 1) * 128],
                            in_=pt.rearrange("p (a b) -> p a b", a=4))

                # ---- condT = c^T . attnT ; out2T = condT + xT (into xT) ----
                for et in range(ET):
                    pm = ps_mm.tile([128, SBLK], F32, tag="mm")
                    for tt in range(TT):
                        nc.tensor.matmul(
                            pm, _r(c_sb[:, tt, et * 128:(et + 1) * 128]),
                            _r(attnT[:, tt, :]), start=(tt == 0), stop=(tt == TT - 1))
                    nc.vector.tensor_add(out=xT[:, et, :], in0=pm, in1=xT[:, et, :])

                # ---- final = out2T^T . W_out' -> DRAM ----
                for j in range(NSUB):
                    outstage = stage.tile([128, H], F32)
                    for hh in range(H // 512):
                        pm = ps_mm.tile([128, 512], F32, tag="mm")
                        for et in range(ET):
                            nc.tensor.matmul(
                                pm, _r(xT[:, et, j * 128:(j + 1) * 128]),
                                _r(wout_sb[:, et, hh * 512:(hh + 1) * 512]),
                                start=(et == 0), stop=(et == ET - 1))
                        nc.scalar.activation(
                            out=outstage[:, hh * 512:(hh + 1) * 512], in_=pm,
                            func=mybir.ActivationFunctionType.Copy)
                    nc.sync.dma_start(
                        out=out_dram[smp, s0 + j * 128: s0 + (j + 1) * 128, :],
                        in_=outstage)

    nc.compile()
    return nc


_NC_CACHE = None


def _get_program():
    global _NC_CACHE
    if _NC_CACHE is None:
        _NC_CACHE = build_program()
    return _NC_CACHE


def kernel(decoderOutput, targetEmbedding_g, encoderOutput_z, c_inputEncoder,
           W_in, b_in, W_out, b_out, _trace=False):
    d = np.ascontiguousarray(np.asarray(decoderOutput, dtype=np.float32))
    g = np.ascontiguousarray(np.asarray(targetEmbedding_g, dtype=np.float32))
    z = np.ascontiguousarray(np.asarray(encoderOutput_z, dtype=np.float32))
    c = np.ascontiguousarray(np.asarray(c_inputEncoder, dtype=np.float32))
    win = np.ascontiguousarray(np.asarray(W_in, dtype=np.float32))
    bin_ = np.asarray(b_in, dtype=np.float32)
    wout = np.asarray(W_out, dtype=np.float32)
    bout = np.asarray(b_out, dtype=np.float32)

    wout_s = np.ascontiguousarray(wout * np.float32(SQRT_HALF))
    bin_t = np.ascontiguousarray(bin_.reshape(ET, 128).T)  # [128, ET]

    nc = _get_program()
    in_maps = []
    for k in range(N_CORES):
        sl = slice(k * BPC, (k + 1) * BPC)
        in_maps.append({
            "d": d[sl], "g": g[sl], "z": z[sl], "c": c[sl],
            "win": win, "wout_s": wout_s, "bin_t": bin_t,
        })
    res = bass_utils.run_bass_kernel_spmd(
        nc, in_maps, core_ids=list(range(N_CORES)), trace=_trace)
    out = np.concatenate([r["out"] for r in res.results], axis=0)
    if bout.any():
        out = out + bout
    kernel.last_results = res
    return out.astype(np.float32)
